# Initial kernel scaffold
#
"""AttentionSortNet (segment-sum -> bucket scores -> gumbel sinkhorn) on 8 trn2 cores.

Self-contained: shards b_h=256 across 8 NeuronCores (32 rows each, data
parallel), runs one Bass/Tile kernel per core via run_bass_kernel_spmd,
gathers the full [256, 64, 64] output.
"""

import os
import sys
from contextlib import ExitStack

import numpy as np

for _p in ("/opt/trn_rl_repo",):
    if _p not in sys.path:
        sys.path.insert(0, _p)

import concourse.bass as bass
import concourse.bacc as bacc
import concourse.tile as tile
from concourse import mybir
from concourse.bass_utils import run_bass_kernel_spmd

# The bacc act-table chooser maps each activation func to the FIRST table
# set containing it; ln->natural_log and exp->exp_and_others ping-pong a
# ~1.3us table reload on every ln<->exp transition. All funcs we use
# (ln/exp/relu/copy) live in natural_log_exp_and_others, so blank every
# other set (preserving dict order == act_func_set_id) to pin the chooser.
_orig_get_act_tables = bacc.get_activation_tables


def _patched_get_act_tables(arch):
    tabs = _orig_get_act_tables(arch)
    if "natural_log_exp_and_others" in tabs:
        for name in list(tabs):
            if name != "natural_log_exp_and_others":
                tabs[name] = set()
    return tabs


bacc.get_activation_tables = _patched_get_act_tables

N_CORES = 8
B_H = 256
T = 4096
D = 64
NB = 64          # buckets
ROWS = B_H // N_CORES   # 32 rows per core
P = 128
KT = T // P      # 32 contraction tiles
EPS = 1e-6
TEMP = 0.7
SCALE = D ** -0.5
ITERS = 8
SGR = 8          # sinkhorn rows per group
NGRP = ROWS // SGR

f32 = mybir.dt.float32
bf16 = mybir.dt.bfloat16
i32 = mybir.dt.int32

_nc_cache: dict[int, "bass.Bass"] = {}


def build(seg_group: int, rows: int = ROWS) -> "bass.Bass":
    AF = mybir.ActivationFunctionType
    OP = mybir.AluOpType
    nseg = (rows + seg_group - 1) // seg_group
    ngrp = (rows + SGR - 1) // SGR
    assert rows % SGR == 0

    nc = bacc.Bacc()
    q_d = nc.declare_dram_parameter("q", [rows, T, D], f32, isOutput=False)
    k_d = nc.declare_dram_parameter("k", [rows, T, D], f32, isOutput=False)
    u_d = nc.declare_dram_parameter("u", [rows, NB, NB], f32, isOutput=False)
    seg_d = nc.declare_dram_parameter("seg", [nseg, T], i32, isOutput=False)
    iota_d = nc.declare_dram_parameter("iota", [P, NB], f32, isOutput=False)
    out_d = nc.declare_dram_parameter("out", [rows, NB, NB], f32, isOutput=True)

    with ExitStack() as ctx:
        tc = ctx.enter_context(tile.TileContext(nc))
        consts = ctx.enter_context(tc.tile_pool(name="consts", bufs=1))
        qkp = ctx.enter_context(tc.tile_pool(name="qkp", bufs=3))
        ohp = ctx.enter_context(tc.tile_pool(name="ohp", bufs=2))
        segp = ctx.enter_context(tc.tile_pool(name="segp", bufs=2))
        rowp = ctx.enter_context(tc.tile_pool(name="rowp", bufs=3))
        rbp = ctx.enter_context(tc.tile_pool(name="rbp", bufs=1))
        snk = ctx.enter_context(tc.tile_pool(name="snk", bufs=2))
        ppq = ctx.enter_context(tc.tile_pool(name="ppq", bufs=2, space="PSUM"))
        ppk = ctx.enter_context(tc.tile_pool(name="ppk", bufs=2, space="PSUM"))
        ppr = ctx.enter_context(tc.tile_pool(name="ppr", bufs=2, space="PSUM"))
        ppc = ctx.enter_context(tc.tile_pool(name="ppc", bufs=2, space="PSUM"))

        iota_sb = consts.tile([P, NB], f32)
        nc.sync.dma_start(out=iota_sb, in_=iota_d[:, :])
        ones64 = consts.tile([NB, NB], f32)
        nc.vector.memset(ones64, 1.0)
        eps64 = consts.tile([NB, 1], f32)
        nc.vector.memset(eps64, EPS)
        # Warm up the ACT ln/exp table set on an instruction with a single
        # sem wait: walrus attaches the table-load to the first user, and a
        # table-load + multi-wait activation exceeds the sync-wait slots.
        warm = consts.tile([NB, 1], f32)
        nc.scalar.activation(out=warm, in_=ones64[:, 0:1], func=AF.Ln)
        nc.scalar.activation(out=warm, in_=warm, func=AF.Exp)

        rbig = [
            rbp.tile([NB, SGR, NB], f32, tag=f"rb{g}", name=f"rbig{g}")
            for g in range(ngrp)
        ]

        def sinkhorn_group(g: int):
            rb = rbig[g]
            rb2 = rb.rearrange("p a b -> p (a b)")
            # r = (log(relu(R/8)+eps) + gumbel) / temperature
            nc.vector.tensor_scalar_mul(out=rb2, in0=rb2, scalar1=1.0 / TEMP)
            for _ in range(ITERS):
                # axis=2 (j, free) logsumexp
                E = snk.tile([NB, SGR, NB], f32, tag="E", name="E")
                nc.scalar.activation(out=E, in_=rb, func=AF.Exp)
                S = snk.tile([NB, SGR], f32, tag="S", name="S")
                nc.vector.tensor_reduce(
                    out=S, in_=E, axis=mybir.AxisListType.X, op=OP.add
                )
                L = snk.tile([NB, SGR], f32, tag="L", name="L")
                nc.scalar.activation(out=L, in_=S, func=AF.Ln)
                for bb in range(SGR):
                    nc.vector.tensor_scalar_sub(
                        out=rb[:, bb, :], in0=rb[:, bb, :], scalar1=L[:, bb : bb + 1]
                    )
                # axis=1 (i, partition) logsumexp: colsums via all-ones matmul,
                # result replicated across all 64 partitions
                E2 = snk.tile([NB, SGR, NB], f32, tag="E2", name="E2")
                nc.scalar.activation(out=E2, in_=rb, func=AF.Exp)
                pc = ppc.tile([NB, SGR * NB], f32, tag="pc", name="pc")
                nc.tensor.matmul(
                    pc,
                    lhsT=ones64,
                    rhs=E2.rearrange("p a b -> p (a b)"),
                    start=True,
                    stop=True,
                )
                Lc = snk.tile([NB, SGR * NB], f32, tag="Lc", name="Lc")
                nc.scalar.activation(out=Lc, in_=pc, func=AF.Ln)
                nc.vector.tensor_sub(out=rb2, in0=rb2, in1=Lc)
            ob = snk.tile([NB, SGR, NB], f32, tag="ob", name="ob")
            nc.scalar.activation(out=ob, in_=rb, func=AF.Exp)
            nc.sync.dma_start(
                out=out_d[g * SGR : (g + 1) * SGR].rearrange("r i j -> i r j"),
                in_=ob,
            )

        oh = None
        for r in range(rows):
            q_sb = qkp.tile([P, KT, D], f32, tag="q", name="q_sb")
            nc.sync.dma_start(out=q_sb, in_=q_d[r].rearrange("(p a) d -> p a d", p=P))
            k_sb = qkp.tile([P, KT, D], f32, tag="k", name="k_sb")
            nc.sync.dma_start(out=k_sb, in_=k_d[r].rearrange("(p a) d -> p a d", p=P))
            u_sb = rowp.tile([NB, NB], f32, tag="u", name="u_sb")
            nc.sync.dma_start(out=u_sb, in_=u_d[r])

            if r % seg_group == 0:
                s = r // seg_group
                seg_i = segp.tile([P, KT], i32, tag="segi", name="seg_i")
                nc.sync.dma_start(
                    out=seg_i, in_=seg_d[s].rearrange("(p a) -> p a", p=P)
                )
                segf = segp.tile([P, KT], f32, tag="segf", name="segf")
                nc.vector.tensor_copy(out=segf, in_=seg_i)
                oh = ohp.tile([P, KT, NB], bf16, tag="oh", name="oh")
                for m in range(KT):
                    nc.vector.tensor_scalar(
                        out=oh[:, m, :],
                        in0=iota_sb,
                        scalar1=segf[:, m : m + 1],
                        scalar2=None,
                        op0=OP.is_equal,
                    )

            # segment sums, transposed: psq[d, s] = sum_t q[t, d] * onehot[t, s]
            psq = ppq.tile([D, NB], f32, tag="psq", name="psq")
            for m in range(KT):
                nc.tensor.matmul(
                    psq,
                    lhsT=q_sb[:, m, :],
                    rhs=oh[:, m, :],
                    start=(m == 0),
                    stop=(m == KT - 1),
                )
            psk = ppk.tile([D, NB], f32, tag="psk", name="psk")
            for m in range(KT):
                nc.tensor.matmul(
                    psk,
                    lhsT=k_sb[:, m, :],
                    rhs=oh[:, m, :],
                    start=(m == 0),
                    stop=(m == KT - 1),
                )
            sq = rowp.tile([D, NB], f32, tag="sq", name="sq")
            nc.scalar.copy(out=sq, in_=psq)
            sk = rowp.tile([D, NB], f32, tag="sk", name="sk")
            nc.scalar.copy(out=sk, in_=psk)

            # R[i, j] = sum_d q_sums[i, d] k_sums[j, d]
            pr = ppr.tile([NB, NB], f32, tag="pr", name="pr")
            nc.tensor.matmul(pr, lhsT=sq, rhs=sk, start=True, stop=True)
            rr = rowp.tile([NB, NB], f32, tag="rr", name="rr")
            nc.scalar.activation(out=rr, in_=pr, func=AF.Relu, scale=SCALE)
            rlog = rowp.tile([NB, NB], f32, tag="rlog", name="rlog")
            nc.scalar.activation(out=rlog, in_=rr, func=AF.Ln, bias=eps64)

            # gumbel = -ln(-ln(u+eps)+eps) = -t2
            t1 = rowp.tile([NB, NB], f32, tag="t1", name="t1")
            nc.scalar.activation(out=t1, in_=u_sb, func=AF.Ln, bias=eps64)
            t2 = rowp.tile([NB, NB], f32, tag="t2", name="t2")
            nc.scalar.activation(out=t2, in_=t1, func=AF.Ln, scale=-1.0, bias=eps64)

            g, sl = divmod(r, SGR)
            nc.vector.tensor_sub(out=rbig[g][:, sl, :], in0=rlog, in1=t2)
            if sl == SGR - 1:
                sinkhorn_group(g)

    nc.finalize()
    return nc


def build_v2(
    seg_group: int, rows: int = ROWS, passes: int = 1, variant: str = "full"
) -> "bass.Bass":
    """v2: single-LDWEIGHTS interleaved q|k segment-sum + pair-packed sinkhorn.

    Per contraction tile m, one weight load [128t, 128(dq|dk)] and one matmul
    against the one-hot [128t, 64s] accumulates both q_sums^T (psum partitions
    0:64) and k_sums^T (64:128). The k half is moved to partition base 0 with
    two PE transposes. Sinkhorn packs 2 rows per partition block: [128, 4, 64]
    per 8-row group, with a block-diagonal ones matrix for per-row colsums.
    """
    AF = mybir.ActivationFunctionType
    OP = mybir.AluOpType
    nseg = (rows + seg_group - 1) // seg_group
    # uneven sinkhorn groups: the last-loaded group is smallest so the
    # serial sinkhorn chain after the final DMA is short
    if rows == 32:
        groups = [12, 10, 8, 2]
    else:
        groups = [max(rows - 2, 2)] + ([2] if rows > 2 else [])
    assert sum(groups) == rows and all(gs % 2 == 0 for gs in groups)
    ngrp = len(groups)
    gstart = [sum(groups[:i]) for i in range(ngrp)]

    nc = bacc.Bacc()
    qkh_d = nc.declare_dram_parameter("qkh", [rows, T, 2 * D], bf16, isOutput=False)
    qkl_d = nc.declare_dram_parameter("qkl", [rows, T, 2 * D], bf16, isOutput=False)
    u_d = nc.declare_dram_parameter("u", [rows, NB, NB], f32, isOutput=False)
    seg_d = nc.declare_dram_parameter("seg", [nseg, T], i32, isOutput=False)
    iota_d = nc.declare_dram_parameter("iota", [P, NB], f32, isOutput=False)
    ident_d = nc.declare_dram_parameter("ident", [P, NB], f32, isOutput=False)
    bd_d = nc.declare_dram_parameter("bd", [P, P], f32, isOutput=False)
    out_d = nc.declare_dram_parameter("out", [rows, NB, NB], f32, isOutput=True)

    with ExitStack() as ctx:
        tc = ctx.enter_context(tile.TileContext(nc))
        consts = ctx.enter_context(tc.tile_pool(name="consts", bufs=1))
        qkp = ctx.enter_context(tc.tile_pool(name="qkp", bufs=3))
        ohp = ctx.enter_context(tc.tile_pool(name="ohp", bufs=2))
        segp = ctx.enter_context(tc.tile_pool(name="segp", bufs=2))
        rowp = ctx.enter_context(tc.tile_pool(name="rowp", bufs=3))
        rbp = ctx.enter_context(tc.tile_pool(name="rbp", bufs=1))
        snk = ctx.enter_context(tc.tile_pool(name="snk", bufs=2))
        pps = ctx.enter_context(tc.tile_pool(name="pps", bufs=2, space="PSUM"))
        ppt = ctx.enter_context(tc.tile_pool(name="ppt", bufs=2, space="PSUM"))
        ppr = ctx.enter_context(tc.tile_pool(name="ppr", bufs=2, space="PSUM"))
        ppc = ctx.enter_context(tc.tile_pool(name="ppc", bufs=2, space="PSUM"))

        iota_sb = consts.tile([P, NB], f32)
        nc.sync.dma_start(out=iota_sb, in_=iota_d[:, :])
        ident_sb = consts.tile([P, NB], f32)
        nc.sync.dma_start(out=ident_sb, in_=ident_d[:, :])
        bd_sb = consts.tile([P, P], f32)
        nc.sync.dma_start(out=bd_sb, in_=bd_d[:, :])
        eps128 = consts.tile([P, 1], f32)
        nc.vector.memset(eps128, EPS)
        warm = consts.tile([P, 1], f32)
        nc.scalar.activation(out=warm, in_=eps128, func=AF.Ln)
        nc.scalar.activation(out=warm, in_=warm, func=AF.Exp)

        rbig = []

        def sinkhorn_group(g: int):
            gs2 = groups[g] // 2
            rb = rbig[g]
            nb2 = gs2 * NB
            rb2 = rb.rearrange("p a b -> p (a b)")
            nc.vector.tensor_scalar_mul(out=rb2, in0=rb2, scalar1=1.0 / TEMP)
            for _ in range(ITERS):
                E = snk.tile([P, gs2, NB], f32, tag=f"E{g}", name="E")
                nc.scalar.activation(out=E, in_=rb, func=AF.Exp)
                S = snk.tile([P, gs2], f32, tag=f"S{g}", name="S")
                nc.vector.tensor_reduce(
                    out=S, in_=E, axis=mybir.AxisListType.X, op=OP.add
                )
                L = snk.tile([P, gs2], f32, tag=f"L{g}", name="L")
                nc.scalar.activation(out=L, in_=S, func=AF.Ln)
                for bb in range(gs2):
                    nc.vector.tensor_scalar_sub(
                        out=rb[:, bb, :], in0=rb[:, bb, :], scalar1=L[:, bb : bb + 1]
                    )
                E2 = snk.tile([P, gs2, NB], f32, tag=f"E2{g}", name="E2")
                nc.scalar.activation(out=E2, in_=rb, func=AF.Exp)
                pc = ppc.tile([P, nb2], f32, tag="pc", name="pc")
                nc.tensor.matmul(
                    pc,
                    lhsT=bd_sb,
                    rhs=E2.rearrange("p a b -> p (a b)"),
                    start=True,
                    stop=True,
                )
                Lc = snk.tile([P, nb2], f32, tag=f"Lc{g}", name="Lc")
                nc.scalar.activation(out=Lc, in_=pc, func=AF.Ln)
                nc.vector.tensor_sub(out=rb2, in0=rb2, in1=Lc)
            ob = snk.tile([P, gs2, NB], f32, tag=f"ob{g}", name="ob")
            nc.scalar.activation(out=ob, in_=rb, func=AF.Exp)
            # out row gstart[g] + blk*2 + par lives at ob[par*64 + i, blk, j]
            nc.sync.dma_start(
                out=out_d[gstart[g] : gstart[g] + groups[g]].rearrange(
                    "(blk par) i j -> par i blk j", par=2
                ),
                in_=ob,
            )

        oh = None
        if variant == "dma":
            # DMA-throughput probe: loads only, one tiny consumer per row
            dummy = consts.tile([P, 1], f32)
            for _pass in range(passes):
                for r in range(rows):
                    qkh_sb = qkp.tile([P, KT, 2 * D], bf16, tag="qh", name="qkh_sb")
                    nc.sync.dma_start(
                        out=qkh_sb, in_=qkh_d[r].rearrange("(p a) d -> p a d", p=P)
                    )
                    qkl_sb = qkp.tile([P, KT, 2 * D], bf16, tag="ql", name="qkl_sb")
                    nc.sync.dma_start(
                        out=qkl_sb, in_=qkl_d[r].rearrange("(p a) d -> p a d", p=P)
                    )
                    nc.vector.tensor_reduce(
                        out=dummy,
                        in_=qkh_sb[:, 0, 0:2],
                        axis=mybir.AxisListType.X,
                        op=OP.add,
                    )
            ob0 = consts.tile([P, (groups[0] // 2) * NB], f32)
            nc.vector.memset(ob0, 0.0)
            for g in range(ngrp):
                nc.sync.dma_start(
                    out=out_d[gstart[g] : gstart[g] + groups[g]].rearrange(
                        "(blk par) i j -> par i blk j", par=2
                    ),
                    in_=ob0[:, : (groups[g] // 2) * NB].rearrange(
                        "p (a b) -> p a b", b=NB
                    ),
                )
            passes = 0  # skip the main body; common finalize below

        qk_fix = None
        if variant == "compute":
            # PE-throughput probe: load q|k once, reuse for every row
            qkh_fix = consts.tile([P, KT, 2 * D], bf16)
            nc.sync.dma_start(
                out=qkh_fix, in_=qkh_d[0].rearrange("(p a) d -> p a d", p=P)
            )
            qkl_fix = consts.tile([P, KT, 2 * D], bf16)
            nc.sync.dma_start(
                out=qkl_fix, in_=qkl_d[0].rearrange("(p a) d -> p a d", p=P)
            )
            qk_fix = (qkh_fix, qkl_fix)
        for _pass in range(passes):
          rbig.clear()
          rbig.extend(
              rbp.tile(
                  [P, groups[g] // 2, NB], f32, tag=f"rb{g}", name=f"rbig{g}"
              )
              for g in range(ngrp)
          )
          for r in range(rows):
            # q|k pre-interleaved on host: one fully-contiguous 2MB DMA per
            # row, and each contraction tile's weight slice [128t, 128d] is a
            # single free dim.
            if variant == "compute":
                qkh_sb, qkl_sb = qk_fix
            else:
                qkh_sb = qkp.tile([P, KT, 2 * D], bf16, tag="qh", name="qkh_sb")
                nc.sync.dma_start(
                    out=qkh_sb, in_=qkh_d[r].rearrange("(p a) d -> p a d", p=P)
                )
                qkl_sb = qkp.tile([P, KT, 2 * D], bf16, tag="ql", name="qkl_sb")
                nc.sync.dma_start(
                    out=qkl_sb, in_=qkl_d[r].rearrange("(p a) d -> p a d", p=P)
                )
            h = r % 2
            hs = slice(h * NB, (h + 1) * NB)
            u_sb = rowp.tile([P, NB], f32, tag="u", name="u_sb")
            nc.sync.dma_start(out=u_sb[hs, :], in_=u_d[r])

            if r % seg_group == 0:
                s = r // seg_group
                seg_i = segp.tile([P, KT], i32, tag="segi", name="seg_i")
                nc.sync.dma_start(
                    out=seg_i, in_=seg_d[s].rearrange("(p a) -> p a", p=P)
                )
                segf = segp.tile([P, KT], f32, tag="segf", name="segf")
                nc.vector.tensor_copy(out=segf, in_=seg_i)
                oh = ohp.tile([P, KT, NB], bf16, tag="oh", name="oh")
                for m in range(KT):
                    nc.vector.tensor_scalar(
                        out=oh[:, m, :],
                        in0=iota_sb,
                        scalar1=segf[:, m : m + 1],
                        scalar2=None,
                        op0=OP.is_equal,
                    )

            # [128(dq|dk), 64s] = sum_t qk[t, :]^T onehot[t, :]
            ps = pps.tile([P, NB], f32, tag="ps", name="ps")
            for m in range(KT):
                nc.tensor.matmul(
                    ps,
                    lhsT=qkh_sb[:, m, :],
                    rhs=oh[:, m, :],
                    start=(m == 0),
                    stop=False,
                )
            for m in range(KT):
                nc.tensor.matmul(
                    ps,
                    lhsT=qkl_sb[:, m, :],
                    rhs=oh[:, m, :],
                    start=False,
                    stop=(m == KT - 1),
                )
            sums = rowp.tile([P, NB], f32, tag="sums", name="sums")
            nc.scalar.copy(out=sums, in_=ps)
            # shift k_sums^T from partitions 64:128 to 0:64 via two transposes
            pt1 = ppt.tile([NB, NB], f32, tag="pt", name="pt1")
            nc.tensor.transpose(pt1, in_=sums[NB:P, :], identity=ident_sb[NB:P, :])
            ka = rowp.tile([NB, NB], f32, tag="ka", name="ka")
            nc.scalar.copy(out=ka, in_=pt1)
            pt2 = ppt.tile([NB, NB], f32, tag="pt", name="pt2")
            nc.tensor.transpose(pt2, in_=ka, identity=ident_sb[0:NB, :])
            kt0 = rowp.tile([NB, NB], f32, tag="kt0", name="kt0")
            nc.scalar.copy(out=kt0, in_=pt2)

            # R[i, j] at psum partition base h*64
            pr = ppr.tile([P, NB], f32, tag="pr", name="pr")
            nc.tensor.matmul(
                pr[hs, :],
                lhsT=sums[0:NB, :],
                rhs=kt0,
                start=True,
                stop=True,
                tile_position=(0, h * NB),
            )
            rr = rowp.tile([P, NB], f32, tag="rr", name="rr")
            nc.scalar.activation(out=rr[hs, :], in_=pr[hs, :], func=AF.Relu, scale=SCALE)
            rlog = rowp.tile([P, NB], f32, tag="rlog", name="rlog")
            nc.scalar.activation(
                out=rlog[hs, :], in_=rr[hs, :], func=AF.Ln, bias=eps128[hs, :]
            )
            t1 = rowp.tile([P, NB], f32, tag="t1", name="t1")
            nc.scalar.activation(
                out=t1[hs, :], in_=u_sb[hs, :], func=AF.Ln, bias=eps128[hs, :]
            )
            t2 = rowp.tile([P, NB], f32, tag="t2", name="t2")
            nc.scalar.activation(
                out=t2[hs, :], in_=t1[hs, :], func=AF.Ln, scale=-1.0, bias=eps128[hs, :]
            )
            g = next(i for i in range(ngrp) if r < gstart[i] + groups[i])
            sl = r - gstart[g]
            blk = sl // 2
            nc.vector.tensor_sub(
                out=rbig[g][hs, blk, :], in0=rlog[hs, :], in1=t2[hs, :]
            )
            if sl == groups[g] - 1:
                sinkhorn_group(g)

    nc.finalize()
    return nc


def _get_nc(seg_group: int) -> "bass.Bass":
    if seg_group not in _nc_cache:
        _nc_cache[seg_group] = build_v2(seg_group)
    return _nc_cache[seg_group]


def kernel(q, k, segment_ids, u):
    q = np.ascontiguousarray(q, dtype=np.float32)
    k = np.ascontiguousarray(k, dtype=np.float32)
    u = np.ascontiguousarray(u, dtype=np.float32)
    seg = np.ascontiguousarray(segment_ids, dtype=np.int32)

    # segment ids repeat per head (row = sample*HEADS + head); verify and
    # share the one-hot build across the group when they do.
    seg3 = seg.reshape(-1, 8, T)
    seg_group = 8 if bool((seg3 == seg3[:, :1]).all()) else 1

    nc = _get_nc(seg_group)
    iota = np.tile(np.arange(NB, dtype=np.float32), (P, 1))
    ident = np.tile(np.eye(NB, dtype=np.float32), (2, 1))
    bd = np.kron(np.eye(2, dtype=np.float32), np.ones((NB, NB), np.float32))
    import ml_dtypes

    qk = np.concatenate([q, k], axis=2)  # [B_H, T, 128]
    qkh = qk.astype(ml_dtypes.bfloat16)
    qkl = (qk - qkh.astype(np.float32)).astype(ml_dtypes.bfloat16)
    in_maps = []
    for c in range(N_CORES):
        sl = slice(c * ROWS, (c + 1) * ROWS)
        in_maps.append(
            {
                "qkh": qkh[sl],
                "qkl": qkl[sl],
                "u": u[sl],
                "seg": np.ascontiguousarray(seg[sl][::seg_group]),
                "iota": iota,
                "ident": ident,
                "bd": bd,
            }
        )
    trace = bool(int(os.environ.get("KERNEL_TRACE", "0")))
    res = run_bass_kernel_spmd(nc, in_maps, core_ids=list(range(N_CORES)), trace=trace)
    kernel.last_results = res
    return np.concatenate([res.results[c]["out"] for c in range(N_CORES)], axis=0)


kernel.last_results = None



# revision 2
# speedup vs baseline: 1.3075x; 1.3075x over previous
"""AttentionSortNet (segment-sum -> bucket scores -> gumbel sinkhorn) on 8 trn2 cores.

Self-contained: shards b_h=256 across 8 NeuronCores (32 rows each, data
parallel), runs one Bass/Tile kernel per core via run_bass_kernel_spmd,
gathers the full [256, 64, 64] output.
"""

import os
import sys
from contextlib import ExitStack

import numpy as np

for _p in ("/opt/trn_rl_repo",):
    if _p not in sys.path:
        sys.path.insert(0, _p)

import concourse.bass as bass
import concourse.bacc as bacc
import concourse.tile as tile
from concourse import mybir
from concourse.bass_utils import run_bass_kernel_spmd

# The bacc act-table chooser maps each activation func to the FIRST table
# set containing it; ln->natural_log and exp->exp_and_others ping-pong a
# ~1.3us table reload on every ln<->exp transition. All funcs we use
# (ln/exp/relu/copy) live in natural_log_exp_and_others, so blank every
# other set (preserving dict order == act_func_set_id) to pin the chooser.
_orig_get_act_tables = bacc.get_activation_tables


def _patched_get_act_tables(arch):
    tabs = _orig_get_act_tables(arch)
    if "natural_log_exp_and_others" in tabs:
        for name in list(tabs):
            if name != "natural_log_exp_and_others":
                tabs[name] = set()
    return tabs


bacc.get_activation_tables = _patched_get_act_tables

N_CORES = 8
B_H = 256
T = 4096
D = 64
NB = 64          # buckets
ROWS = B_H // N_CORES   # 32 rows per core
P = 128
KT = T // P      # 32 contraction tiles
EPS = 1e-6
TEMP = 0.7
SCALE = D ** -0.5
ITERS = 8
SGR = 8          # sinkhorn rows per group
NGRP = ROWS // SGR

f32 = mybir.dt.float32
bf16 = mybir.dt.bfloat16
i32 = mybir.dt.int32

_nc_cache: dict[int, "bass.Bass"] = {}


def build(seg_group: int, rows: int = ROWS) -> "bass.Bass":
    AF = mybir.ActivationFunctionType
    OP = mybir.AluOpType
    nseg = (rows + seg_group - 1) // seg_group
    ngrp = (rows + SGR - 1) // SGR
    assert rows % SGR == 0

    nc = bacc.Bacc()
    q_d = nc.declare_dram_parameter("q", [rows, T, D], f32, isOutput=False)
    k_d = nc.declare_dram_parameter("k", [rows, T, D], f32, isOutput=False)
    u_d = nc.declare_dram_parameter("u", [rows, NB, NB], f32, isOutput=False)
    seg_d = nc.declare_dram_parameter("seg", [nseg, T], i32, isOutput=False)
    iota_d = nc.declare_dram_parameter("iota", [P, NB], f32, isOutput=False)
    out_d = nc.declare_dram_parameter("out", [rows, NB, NB], f32, isOutput=True)

    with ExitStack() as ctx:
        tc = ctx.enter_context(tile.TileContext(nc))
        consts = ctx.enter_context(tc.tile_pool(name="consts", bufs=1))
        qkp = ctx.enter_context(tc.tile_pool(name="qkp", bufs=3))
        ohp = ctx.enter_context(tc.tile_pool(name="ohp", bufs=2))
        segp = ctx.enter_context(tc.tile_pool(name="segp", bufs=2))
        rowp = ctx.enter_context(tc.tile_pool(name="rowp", bufs=3))
        rbp = ctx.enter_context(tc.tile_pool(name="rbp", bufs=1))
        snk = ctx.enter_context(tc.tile_pool(name="snk", bufs=2))
        ppq = ctx.enter_context(tc.tile_pool(name="ppq", bufs=2, space="PSUM"))
        ppk = ctx.enter_context(tc.tile_pool(name="ppk", bufs=2, space="PSUM"))
        ppr = ctx.enter_context(tc.tile_pool(name="ppr", bufs=2, space="PSUM"))
        ppc = ctx.enter_context(tc.tile_pool(name="ppc", bufs=2, space="PSUM"))

        iota_sb = consts.tile([P, NB], f32)
        nc.sync.dma_start(out=iota_sb, in_=iota_d[:, :])
        ones64 = consts.tile([NB, NB], f32)
        nc.vector.memset(ones64, 1.0)
        eps64 = consts.tile([NB, 1], f32)
        nc.vector.memset(eps64, EPS)
        # Warm up the ACT ln/exp table set on an instruction with a single
        # sem wait: walrus attaches the table-load to the first user, and a
        # table-load + multi-wait activation exceeds the sync-wait slots.
        warm = consts.tile([NB, 1], f32)
        nc.scalar.activation(out=warm, in_=ones64[:, 0:1], func=AF.Ln)
        nc.scalar.activation(out=warm, in_=warm, func=AF.Exp)

        rbig = [
            rbp.tile([NB, SGR, NB], f32, tag=f"rb{g}", name=f"rbig{g}")
            for g in range(ngrp)
        ]

        def sinkhorn_group(g: int):
            rb = rbig[g]
            rb2 = rb.rearrange("p a b -> p (a b)")
            # r = (log(relu(R/8)+eps) + gumbel) / temperature
            nc.vector.tensor_scalar_mul(out=rb2, in0=rb2, scalar1=1.0 / TEMP)
            for _ in range(ITERS):
                # axis=2 (j, free) logsumexp
                E = snk.tile([NB, SGR, NB], f32, tag="E", name="E")
                nc.scalar.activation(out=E, in_=rb, func=AF.Exp)
                S = snk.tile([NB, SGR], f32, tag="S", name="S")
                nc.vector.tensor_reduce(
                    out=S, in_=E, axis=mybir.AxisListType.X, op=OP.add
                )
                L = snk.tile([NB, SGR], f32, tag="L", name="L")
                nc.scalar.activation(out=L, in_=S, func=AF.Ln)
                for bb in range(SGR):
                    nc.vector.tensor_scalar_sub(
                        out=rb[:, bb, :], in0=rb[:, bb, :], scalar1=L[:, bb : bb + 1]
                    )
                # axis=1 (i, partition) logsumexp: colsums via all-ones matmul,
                # result replicated across all 64 partitions
                E2 = snk.tile([NB, SGR, NB], f32, tag="E2", name="E2")
                nc.scalar.activation(out=E2, in_=rb, func=AF.Exp)
                pc = ppc.tile([NB, SGR * NB], f32, tag="pc", name="pc")
                nc.tensor.matmul(
                    pc,
                    lhsT=ones64,
                    rhs=E2.rearrange("p a b -> p (a b)"),
                    start=True,
                    stop=True,
                )
                Lc = snk.tile([NB, SGR * NB], f32, tag="Lc", name="Lc")
                nc.scalar.activation(out=Lc, in_=pc, func=AF.Ln)
                nc.vector.tensor_sub(out=rb2, in0=rb2, in1=Lc)
            ob = snk.tile([NB, SGR, NB], f32, tag="ob", name="ob")
            nc.scalar.activation(out=ob, in_=rb, func=AF.Exp)
            nc.sync.dma_start(
                out=out_d[g * SGR : (g + 1) * SGR].rearrange("r i j -> i r j"),
                in_=ob,
            )

        oh = None
        for r in range(rows):
            q_sb = qkp.tile([P, KT, D], f32, tag="q", name="q_sb")
            nc.sync.dma_start(out=q_sb, in_=q_d[r].rearrange("(p a) d -> p a d", p=P))
            k_sb = qkp.tile([P, KT, D], f32, tag="k", name="k_sb")
            nc.sync.dma_start(out=k_sb, in_=k_d[r].rearrange("(p a) d -> p a d", p=P))
            u_sb = rowp.tile([NB, NB], f32, tag="u", name="u_sb")
            nc.sync.dma_start(out=u_sb, in_=u_d[r])

            if r % seg_group == 0:
                s = r // seg_group
                seg_i = segp.tile([P, KT], i32, tag="segi", name="seg_i")
                nc.sync.dma_start(
                    out=seg_i, in_=seg_d[s].rearrange("(p a) -> p a", p=P)
                )
                segf = segp.tile([P, KT], f32, tag="segf", name="segf")
                nc.vector.tensor_copy(out=segf, in_=seg_i)
                oh = ohp.tile([P, KT, NB], bf16, tag="oh", name="oh")
                for m in range(KT):
                    nc.vector.tensor_scalar(
                        out=oh[:, m, :],
                        in0=iota_sb,
                        scalar1=segf[:, m : m + 1],
                        scalar2=None,
                        op0=OP.is_equal,
                    )

            # segment sums, transposed: psq[d, s] = sum_t q[t, d] * onehot[t, s]
            psq = ppq.tile([D, NB], f32, tag="psq", name="psq")
            for m in range(KT):
                nc.tensor.matmul(
                    psq,
                    lhsT=q_sb[:, m, :],
                    rhs=oh[:, m, :],
                    start=(m == 0),
                    stop=(m == KT - 1),
                )
            psk = ppk.tile([D, NB], f32, tag="psk", name="psk")
            for m in range(KT):
                nc.tensor.matmul(
                    psk,
                    lhsT=k_sb[:, m, :],
                    rhs=oh[:, m, :],
                    start=(m == 0),
                    stop=(m == KT - 1),
                )
            sq = rowp.tile([D, NB], f32, tag="sq", name="sq")
            nc.scalar.copy(out=sq, in_=psq)
            sk = rowp.tile([D, NB], f32, tag="sk", name="sk")
            nc.scalar.copy(out=sk, in_=psk)

            # R[i, j] = sum_d q_sums[i, d] k_sums[j, d]
            pr = ppr.tile([NB, NB], f32, tag="pr", name="pr")
            nc.tensor.matmul(pr, lhsT=sq, rhs=sk, start=True, stop=True)
            rr = rowp.tile([NB, NB], f32, tag="rr", name="rr")
            nc.scalar.activation(out=rr, in_=pr, func=AF.Relu, scale=SCALE)
            rlog = rowp.tile([NB, NB], f32, tag="rlog", name="rlog")
            nc.scalar.activation(out=rlog, in_=rr, func=AF.Ln, bias=eps64)

            # gumbel = -ln(-ln(u+eps)+eps) = -t2
            t1 = rowp.tile([NB, NB], f32, tag="t1", name="t1")
            nc.scalar.activation(out=t1, in_=u_sb, func=AF.Ln, bias=eps64)
            t2 = rowp.tile([NB, NB], f32, tag="t2", name="t2")
            nc.scalar.activation(out=t2, in_=t1, func=AF.Ln, scale=-1.0, bias=eps64)

            g, sl = divmod(r, SGR)
            nc.vector.tensor_sub(out=rbig[g][:, sl, :], in0=rlog, in1=t2)
            if sl == SGR - 1:
                sinkhorn_group(g)

    nc.finalize()
    return nc


def build_v2(
    seg_group: int, rows: int = ROWS, passes: int = 1, variant: str = "full"
) -> "bass.Bass":
    """v2: single-LDWEIGHTS interleaved q|k segment-sum + pair-packed sinkhorn.

    Per contraction tile m, one weight load [128t, 128(dq|dk)] and one matmul
    against the one-hot [128t, 64s] accumulates both q_sums^T (psum partitions
    0:64) and k_sums^T (64:128). The k half is moved to partition base 0 with
    two PE transposes. Sinkhorn packs 2 rows per partition block: [128, 4, 64]
    per 8-row group, with a block-diagonal ones matrix for per-row colsums.
    """
    AF = mybir.ActivationFunctionType
    OP = mybir.AluOpType
    nseg = (rows + seg_group - 1) // seg_group
    # uneven sinkhorn groups: the last-loaded group is smallest so the
    # serial sinkhorn chain after the final DMA is short
    if rows == 32:
        groups = [12, 10, 8, 2]
    else:
        groups = [max(rows - 2, 2)] + ([2] if rows > 2 else [])
    assert sum(groups) == rows and all(gs % 2 == 0 for gs in groups)
    ngrp = len(groups)
    gstart = [sum(groups[:i]) for i in range(ngrp)]

    nc = bacc.Bacc()
    qkh_d = nc.declare_dram_parameter("qkh", [rows, T, 2 * D], bf16, isOutput=False)
    qkl_d = nc.declare_dram_parameter("qkl", [rows, T, 2 * D], bf16, isOutput=False)
    u_d = nc.declare_dram_parameter("u", [rows, NB, NB], f32, isOutput=False)
    seg_d = nc.declare_dram_parameter("seg", [nseg, T], i32, isOutput=False)
    iota_d = nc.declare_dram_parameter("iota", [P, NB], f32, isOutput=False)
    ident_d = nc.declare_dram_parameter("ident", [P, NB], f32, isOutput=False)
    bd_d = nc.declare_dram_parameter("bd", [P, P], f32, isOutput=False)
    out_d = nc.declare_dram_parameter("out", [rows, NB, NB], f32, isOutput=True)

    with ExitStack() as ctx:
        tc = ctx.enter_context(tile.TileContext(nc))
        consts = ctx.enter_context(tc.tile_pool(name="consts", bufs=1))
        qkp = ctx.enter_context(tc.tile_pool(name="qkp", bufs=3))
        ohp = ctx.enter_context(tc.tile_pool(name="ohp", bufs=2))
        segp = ctx.enter_context(tc.tile_pool(name="segp", bufs=2))
        rowp = ctx.enter_context(tc.tile_pool(name="rowp", bufs=3))
        rbp = ctx.enter_context(tc.tile_pool(name="rbp", bufs=1))
        snk = ctx.enter_context(tc.tile_pool(name="snk", bufs=2))
        pps = ctx.enter_context(tc.tile_pool(name="pps", bufs=2, space="PSUM"))
        ppt = ctx.enter_context(tc.tile_pool(name="ppt", bufs=2, space="PSUM"))
        ppr = ctx.enter_context(tc.tile_pool(name="ppr", bufs=2, space="PSUM"))
        ppc = ctx.enter_context(tc.tile_pool(name="ppc", bufs=2, space="PSUM"))

        iota_sb = consts.tile([P, NB], f32)
        nc.sync.dma_start(out=iota_sb, in_=iota_d[:, :])
        ident_sb = consts.tile([P, NB], f32)
        nc.sync.dma_start(out=ident_sb, in_=ident_d[:, :])
        bd_sb = consts.tile([P, P], f32)
        nc.sync.dma_start(out=bd_sb, in_=bd_d[:, :])
        eps128 = consts.tile([P, 1], f32)
        nc.vector.memset(eps128, EPS)
        warm = consts.tile([P, 1], f32)
        nc.scalar.activation(out=warm, in_=eps128, func=AF.Ln)
        nc.scalar.activation(out=warm, in_=warm, func=AF.Exp)

        rbig = []

        def sinkhorn_group(g: int):
            gs2 = groups[g] // 2
            rb = rbig[g]
            nb2 = gs2 * NB
            rb2 = rb.rearrange("p a b -> p (a b)")
            nc.vector.tensor_scalar_mul(out=rb2, in0=rb2, scalar1=1.0 / TEMP)
            for _ in range(ITERS):
                E = snk.tile([P, gs2, NB], f32, tag=f"E{g}", name="E")
                nc.scalar.activation(out=E, in_=rb, func=AF.Exp)
                S = snk.tile([P, gs2], f32, tag=f"S{g}", name="S")
                nc.vector.tensor_reduce(
                    out=S, in_=E, axis=mybir.AxisListType.X, op=OP.add
                )
                L = snk.tile([P, gs2], f32, tag=f"L{g}", name="L")
                nc.scalar.activation(out=L, in_=S, func=AF.Ln)
                for bb in range(gs2):
                    nc.vector.tensor_scalar_sub(
                        out=rb[:, bb, :], in0=rb[:, bb, :], scalar1=L[:, bb : bb + 1]
                    )
                E2 = snk.tile([P, gs2, NB], f32, tag=f"E2{g}", name="E2")
                nc.scalar.activation(out=E2, in_=rb, func=AF.Exp)
                pc = ppc.tile([P, nb2], f32, tag="pc", name="pc")
                nc.tensor.matmul(
                    pc,
                    lhsT=bd_sb,
                    rhs=E2.rearrange("p a b -> p (a b)"),
                    start=True,
                    stop=True,
                )
                Lc = snk.tile([P, nb2], f32, tag=f"Lc{g}", name="Lc")
                nc.scalar.activation(out=Lc, in_=pc, func=AF.Ln)
                nc.vector.tensor_sub(out=rb2, in0=rb2, in1=Lc)
            ob = snk.tile([P, gs2, NB], f32, tag=f"ob{g}", name="ob")
            nc.scalar.activation(out=ob, in_=rb, func=AF.Exp)
            # out row gstart[g] + blk*2 + par lives at ob[par*64 + i, blk, j]
            nc.sync.dma_start(
                out=out_d[gstart[g] : gstart[g] + groups[g]].rearrange(
                    "(blk par) i j -> par i blk j", par=2
                ),
                in_=ob,
            )

        oh = None
        if variant == "dma":
            # DMA-throughput probe: loads only, one tiny consumer per row
            dummy = consts.tile([P, 1], f32)
            for _pass in range(passes):
                for r in range(rows):
                    qkh_sb = qkp.tile([P, KT, 2 * D], bf16, tag="qh", name="qkh_sb")
                    nc.sync.dma_start(
                        out=qkh_sb, in_=qkh_d[r].rearrange("(p a) d -> p a d", p=P)
                    )
                    qkl_sb = qkp.tile([P, KT, 2 * D], bf16, tag="ql", name="qkl_sb")
                    nc.sync.dma_start(
                        out=qkl_sb, in_=qkl_d[r].rearrange("(p a) d -> p a d", p=P)
                    )
                    nc.vector.tensor_reduce(
                        out=dummy,
                        in_=qkh_sb[:, 0, 0:2],
                        axis=mybir.AxisListType.X,
                        op=OP.add,
                    )
            ob0 = consts.tile([P, (groups[0] // 2) * NB], f32)
            nc.vector.memset(ob0, 0.0)
            for g in range(ngrp):
                nc.sync.dma_start(
                    out=out_d[gstart[g] : gstart[g] + groups[g]].rearrange(
                        "(blk par) i j -> par i blk j", par=2
                    ),
                    in_=ob0[:, : (groups[g] // 2) * NB].rearrange(
                        "p (a b) -> p a b", b=NB
                    ),
                )
            passes = 0  # skip the main body; common finalize below

        qk_fix = None
        if variant == "compute":
            # PE-throughput probe: load q|k once, reuse for every row
            qkh_fix = consts.tile([P, KT, 2 * D], bf16)
            nc.sync.dma_start(
                out=qkh_fix, in_=qkh_d[0].rearrange("(p a) d -> p a d", p=P)
            )
            qkl_fix = consts.tile([P, KT, 2 * D], bf16)
            nc.sync.dma_start(
                out=qkl_fix, in_=qkl_d[0].rearrange("(p a) d -> p a d", p=P)
            )
            qk_fix = (qkh_fix, qkl_fix)
        for _pass in range(passes):
          rbig.clear()
          rbig.extend(
              rbp.tile(
                  [P, groups[g] // 2, NB], f32, tag=f"rb{g}", name=f"rbig{g}"
              )
              for g in range(ngrp)
          )
          for r in range(rows):
            # q|k pre-interleaved on host: one fully-contiguous 2MB DMA per
            # row, and each contraction tile's weight slice [128t, 128d] is a
            # single free dim.
            if variant == "compute":
                qkh_sb, qkl_sb = qk_fix
            else:
                qkh_sb = qkp.tile([P, KT, 2 * D], bf16, tag="qh", name="qkh_sb")
                nc.sync.dma_start(
                    out=qkh_sb, in_=qkh_d[r].rearrange("(p a) d -> p a d", p=P)
                )
                qkl_sb = qkp.tile([P, KT, 2 * D], bf16, tag="ql", name="qkl_sb")
                nc.sync.dma_start(
                    out=qkl_sb, in_=qkl_d[r].rearrange("(p a) d -> p a d", p=P)
                )
            h = r % 2
            hs = slice(h * NB, (h + 1) * NB)
            u_sb = rowp.tile([P, NB], f32, tag="u", name="u_sb")
            nc.sync.dma_start(out=u_sb[hs, :], in_=u_d[r])

            if r % seg_group == 0:
                s = r // seg_group
                seg_i = segp.tile([P, KT], i32, tag="segi", name="seg_i")
                nc.sync.dma_start(
                    out=seg_i, in_=seg_d[s].rearrange("(p a) -> p a", p=P)
                )
                segf = segp.tile([P, KT], f32, tag="segf", name="segf")
                nc.vector.tensor_copy(out=segf, in_=seg_i)
                oh = ohp.tile([P, KT, NB], bf16, tag="oh", name="oh")
                for m in range(KT):
                    nc.vector.tensor_scalar(
                        out=oh[:, m, :],
                        in0=iota_sb,
                        scalar1=segf[:, m : m + 1],
                        scalar2=None,
                        op0=OP.is_equal,
                    )

            # [128(dq|dk), 64s] = sum_t qk[t, :]^T onehot[t, :]
            ps = pps.tile([P, NB], f32, tag="ps", name="ps")
            for m in range(KT):
                nc.tensor.matmul(
                    ps,
                    lhsT=qkh_sb[:, m, :],
                    rhs=oh[:, m, :],
                    start=(m == 0),
                    stop=False,
                )
            for m in range(KT):
                nc.tensor.matmul(
                    ps,
                    lhsT=qkl_sb[:, m, :],
                    rhs=oh[:, m, :],
                    start=False,
                    stop=(m == KT - 1),
                )
            sums = rowp.tile([P, NB], f32, tag="sums", name="sums")
            nc.scalar.copy(out=sums, in_=ps)
            # shift k_sums^T from partitions 64:128 to 0:64 via two transposes
            pt1 = ppt.tile([NB, NB], f32, tag="pt", name="pt1")
            nc.tensor.transpose(pt1, in_=sums[NB:P, :], identity=ident_sb[NB:P, :])
            ka = rowp.tile([NB, NB], f32, tag="ka", name="ka")
            nc.scalar.copy(out=ka, in_=pt1)
            pt2 = ppt.tile([NB, NB], f32, tag="pt", name="pt2")
            nc.tensor.transpose(pt2, in_=ka, identity=ident_sb[0:NB, :])
            kt0 = rowp.tile([NB, NB], f32, tag="kt0", name="kt0")
            nc.scalar.copy(out=kt0, in_=pt2)

            # R[i, j] at psum partition base h*64
            pr = ppr.tile([P, NB], f32, tag="pr", name="pr")
            nc.tensor.matmul(
                pr[hs, :],
                lhsT=sums[0:NB, :],
                rhs=kt0,
                start=True,
                stop=True,
                tile_position=(0, h * NB),
            )
            rr = rowp.tile([P, NB], f32, tag="rr", name="rr")
            nc.scalar.activation(out=rr[hs, :], in_=pr[hs, :], func=AF.Relu, scale=SCALE)
            rlog = rowp.tile([P, NB], f32, tag="rlog", name="rlog")
            nc.scalar.activation(
                out=rlog[hs, :], in_=rr[hs, :], func=AF.Ln, bias=eps128[hs, :]
            )
            t1 = rowp.tile([P, NB], f32, tag="t1", name="t1")
            nc.scalar.activation(
                out=t1[hs, :], in_=u_sb[hs, :], func=AF.Ln, bias=eps128[hs, :]
            )
            t2 = rowp.tile([P, NB], f32, tag="t2", name="t2")
            nc.scalar.activation(
                out=t2[hs, :], in_=t1[hs, :], func=AF.Ln, scale=-1.0, bias=eps128[hs, :]
            )
            g = next(i for i in range(ngrp) if r < gstart[i] + groups[i])
            sl = r - gstart[g]
            blk = sl // 2
            nc.vector.tensor_sub(
                out=rbig[g][hs, blk, :], in0=rlog[hs, :], in1=t2[hs, :]
            )
            if sl == groups[g] - 1:
                sinkhorn_group(g)

    nc.finalize()
    return nc


def _get_nc(seg_group: int) -> "bass.Bass":
    if seg_group not in _nc_cache:
        _nc_cache[seg_group] = build_v2(seg_group)
    return _nc_cache[seg_group]


def build_program(seg_group: int = 8, passes: int = 1, variant: str = "full"):
    return build_v2(seg_group, passes=passes, variant=variant)


def prep_in_maps(q, k, segment_ids, u):
    q = np.ascontiguousarray(q, dtype=np.float32)
    k = np.ascontiguousarray(k, dtype=np.float32)
    u = np.ascontiguousarray(u, dtype=np.float32)
    seg = np.ascontiguousarray(segment_ids, dtype=np.int32)

    # segment ids repeat per head (row = sample*HEADS + head); verify and
    # share the one-hot build across the group when they do.
    seg3 = seg.reshape(-1, 8, T)
    seg_group = 8 if bool((seg3 == seg3[:, :1]).all()) else 1

    iota = np.tile(np.arange(NB, dtype=np.float32), (P, 1))
    ident = np.tile(np.eye(NB, dtype=np.float32), (2, 1))
    bd = np.kron(np.eye(2, dtype=np.float32), np.ones((NB, NB), np.float32))
    import ml_dtypes

    qk = np.concatenate([q, k], axis=2)  # [B_H, T, 128]
    qkh = qk.astype(ml_dtypes.bfloat16)
    qkl = (qk - qkh.astype(np.float32)).astype(ml_dtypes.bfloat16)
    in_maps = []
    for c in range(N_CORES):
        sl = slice(c * ROWS, (c + 1) * ROWS)
        in_maps.append(
            {
                "qkh": qkh[sl],
                "qkl": qkl[sl],
                "u": u[sl],
                "seg": np.ascontiguousarray(seg[sl][::seg_group]),
                "iota": iota,
                "ident": ident,
                "bd": bd,
            }
        )
    return in_maps, seg_group


def kernel(q, k, segment_ids, u):
    in_maps, seg_group = prep_in_maps(q, k, segment_ids, u)
    nc = _get_nc(seg_group)
    trace = bool(int(os.environ.get("KERNEL_TRACE", "0")))
    res = run_bass_kernel_spmd(nc, in_maps, core_ids=list(range(N_CORES)), trace=trace)
    kernel.last_results = res
    return np.concatenate([res.results[c]["out"] for c in range(N_CORES)], axis=0)


kernel.last_results = None

